# revision 32
# baseline (speedup 1.0000x reference)
"""Channel-wise Linear on 8 TRN2 NeuronCores.

y[b, c, :] = x[b, c, :] @ W[c].T + b[c]   (B=64, C=128, F=1024, fp32 ref)

Sharding: channels split across 8 cores (16 each, expert-style), no
cross-core communication.

Numerics: traffic is the whole story (W would be 32 MB/core in bf16), so
both matmul operands ship as float8 e3m4 (4 mantissa bits): W scaled by
32 and x by 2 so the randn-scaled values sit in the e3m4 normal range
(max 15.5, no clipping). Measured end-to-end rel err 1.61e-2 vs the
2e-2 gate (e4m3's 3-bit mantissa fails at 2.3e-2). PSUM accumulates in
fp32; the 1/64 rescale rides the scalar slot of the PSUM-evacuation op.
Output leaves as fp16.

Device, per channel: 8 K-tiles of xT.T @ WT accumulate into one 2-bank
PSUM tile (x stationary, W moving at 1 col/cycle; LDWEIGHTS of the next
k-tile overlaps the current matmul, verified on HW). Bias never touches
the PE: bias rows are partition-broadcast on gpsimd, and PSUM->SBUF
evacuation is a fused (psum*1/64 + bias) scalar_tensor_tensor on the
vector engine - two 512-col ops so the first overlaps the second half's
last matmuls - packing 4 channels side by side into a [64, 4F] fp16
tile (8 KB/partition descriptors). Outputs go out on the scalar
engine's HW-DGE queue, off the 16 sync-engine queues that stream W;
the last group flushes in progressively finer pieces so the final DMA
is small and fires immediately after channel 15's evacuation.

Per-core traffic: 16.8 MB W + 1.05 MB x + 2.1 MB y ~ 20 MB (~55 us at
the ~360 GB/s/core the 16 shared DMA engines sustain), PE: 16ch x 8kt
x 1024 cols = 131k cycles (~55 us at 2.4 GHz) - right at the ridge, so
exec is DMA-paced and run-to-run HBM contention shows as +-4 us.

Schedule notes (from perfetto/NTFF traces):
- HAM clock-gates the PE to 4/8 until ~4 us of SUSTAINED wide matmul
  activity; tiny seed matmuls don't ramp it, and a >~2.5 us idle drops
  it back. The warm-up chain of full-array matmuls is sized to end just
  as channel 0's first W chunk lands (~13 us), handing off seamlessly.
- DMA takes ~7 us after kernel start to begin flowing (engine-boot
  barriers), then ~5 us to ramp to full rate; channel 0's x chunk and a
  small leading W chunk are enqueued first so kt0 starts earliest.
- The last ~6 us is NEFF teardown (per-engine event-semaphore resets +
  two cross-core barriers) emitted by codegen - not addressable from
  kernel code; the Tile-context sem-clear chain is patched out below.
"""

import numpy as np
import ml_dtypes

import concourse.bass as bass
import concourse.bacc as bacc
import concourse.mybir as mybir
from concourse import tile
from concourse import bass_utils

B, C, F = 64, 128, 1024
NCORES = 8
CPC = C // NCORES          # channels per core
KT = F // 128              # contraction tiles per channel
F32 = mybir.dt.float32
BF16 = mybir.dt.bfloat16
FP16 = mybir.dt.float16
FP8 = mybir.dt.float8e3    # e3m4
WSCALE = 32.0
XSCALE = 2.0
OSCALE = 1.0 / (WSCALE * XSCALE)
WARMUP_MM = 15             # warm-up matmuls (~0.4-0.8 us each while ramping)

_CACHE = {}


def _patch_fast_teardown():
    """Skip the end-of-kernel semaphore-clear chain (~5-8 us of serial
    EVENT_SEMAPHORE resets + an extra all-engine barrier). The NEFF here is
    loaded fresh per execution, so post-run semaphore hygiene only adds tail
    latency. Allocator bookkeeping (freeing the sem ids) is preserved by
    calling the original helper with instruction emission suppressed."""
    from concourse import tile as _t

    if getattr(_t.TileContext._drain_and_barrier, "_fast_teardown", False):
        return

    def _drain_and_barrier(self, tick_clock, wait_clock):
        drain_inst = self.nc.sync.drain()
        wait_clock.add_sem_waits(
            drain_inst.ins, _t.ScopedClock({None: tick_clock.global_clock})
        )
        self.nc.all_engine_barrier()
        assert self.sems is not None
        popped = self.nc._tile_sem_poison_stack.pop()
        assert popped is self._sem_poison
        gp = self.nc.gpsimd
        orig = (gp.dma_reset, gp.sem_clear)
        try:
            gp.dma_reset = lambda *a, **k: None
            gp.sem_clear = lambda *a, **k: None
        except AttributeError:
            self.nc.clear_and_free_semaphores(list(self.sems.allocated().values()))
            self.nc.all_engine_barrier()
            return
        try:
            self.nc.clear_and_free_semaphores(list(self.sems.allocated().values()))
        finally:
            gp.dma_reset, gp.sem_clear = orig

    _drain_and_barrier._fast_teardown = True
    _t.TileContext._drain_and_barrier = _drain_and_barrier


def _build():
    if "nc" in _CACHE:
        return _CACHE["nc"]
    _patch_fast_teardown()
    nc = bacc.Bacc(
        "TRN2",
        target_bir_lowering=False,
        debug=False,
        enable_asserts=False,
        num_devices=NCORES,
    )
    wh = nc.dram_tensor("wh", [CPC, 128, KT * F], FP8, kind="ExternalInput").ap()
    xs = nc.dram_tensor("xs", [128, CPC * KT * B], FP8, kind="ExternalInput").ap()
    bs = nc.dram_tensor("bs", [1, CPC * F], FP16, kind="ExternalInput").ap()
    # channels 0-7 leave as fp16; channels 8-15 as fp8 e3m4 at 2x scale
    # (host halves them back) - rel err 1.86e-2 vs the 2e-2 gate, and 0.5 MB
    # less output traffic per core
    yc16 = nc.dram_tensor("yc16", [2, B, 4 * F], FP16, kind="ExternalOutput").ap()
    yc8 = nc.dram_tensor("yc8", [2, B, 4 * F], FP8, kind="ExternalOutput").ap()

    with tile.TileContext(nc) as tc:
        with (
            tc.tile_pool(name="w", bufs=12) as wpool,
            tc.tile_pool(name="x", bufs=1) as xpool,
            tc.tile_pool(name="bi", bufs=1) as bpool,
            tc.tile_pool(name="wa", bufs=1) as wupool,
            tc.tile_pool(name="o", bufs=3) as opool,
            tc.tile_pool(name="ps", bufs=4, space=bass.MemorySpace.PSUM) as pspool,
        ):
            # PE warm-up: full-array matmuls on junk data keep the PE busy
            # while channel 0's W streams in; HAM only ramps the PE clock to
            # 8/8 after ~6 us of SUSTAINED wide matmul activity (tiny seed
            # matmuls don't ramp it), so make the warm-up look like real work.
            wa = wupool.tile([128, 512], BF16)
            nc.gpsimd.memset(wa[:], 1.0)
            wu = pspool.tile([128, 512], F32, tag="ps")
            for _ in range(WARMUP_MM):
                nc.tensor.matmul(
                    wu[:], wa[:, 0:128], wa[:], start=True, stop=True,
                    skip_group_check=True,
                )

            b_sb = bpool.tile([1, CPC * F], FP16)
            bb = bpool.tile([B, CPC * F], FP16)

            # x prefetched one 4-channel group ahead of its W stream so W
            # never queues behind bulk x traffic on the HW-DGE queues
            x_all = xpool.tile([128, CPC * KT * B], FP8)
            xq = CPC * KT * B // 4

            o_t = None
            for c in range(CPC):
                w_t = wpool.tile([128, KT * F], FP8)
                half = KT * F // 2
                if c == 0:
                    # channel 0 owns the critical path: its own tiny x chunk
                    # and a small leading W chunk first so kt0 starts early
                    xc0 = KT * B
                    nc.sync.dma_start(x_all[:, 0:xc0], xs[:, 0:xc0])
                    qf = 2 * F
                    nc.sync.dma_start(w_t[:, 0:qf], wh[c][:, 0:qf])
                    nc.sync.dma_start(x_all[:, xc0:xq], xs[:, xc0:xq])
                    nc.sync.dma_start(w_t[:, qf:half], wh[c][:, qf:half])
                    nc.sync.dma_start(w_t[:, half:], wh[c][:, half:])
                    nc.sync.dma_start(b_sb[:], bs[:])
                elif c < 4 or c >= 12:
                    # fine-granularity W where the PE runs close behind the
                    # DMA: early channels (pipe still building its lead) and
                    # late channels (in DMA-paced phases the tail otherwise
                    # waits on the whole final 1 MB before any matmul)
                    nq = 2 if c < 14 else 4
                    st = KT * F // nq
                    for j in range(nq):
                        nc.sync.dma_start(
                            w_t[:, j * st:(j + 1) * st], wh[c][:, j * st:(j + 1) * st]
                        )
                else:
                    # steady state: one dma_start per channel (8 KB/partition
                    # descriptors), prefetch pool hides the coarser signal
                    nc.sync.dma_start(w_t[:], wh[c])
                # bias rows broadcast over the batch partitions on gpsimd
                nc.gpsimd.partition_broadcast(
                    bb[:, c * F:(c + 1) * F], b_sb[:, c * F:(c + 1) * F], channels=B
                )
                if c % 4 == 0 and c + 4 < CPC:
                    j = c // 4 + 1
                    nc.sync.dma_start(
                        x_all[:, j * xq:(j + 1) * xq], xs[:, j * xq:(j + 1) * xq]
                    )

                # one 2-bank PSUM tile per channel: a single 1024-col
                # evacuation op instead of two
                ps = pspool.tile([B, 1024], F32, tag="ps")
                for kt in range(KT):
                    lhsT = x_all[:, (c * KT + kt) * B:(c * KT + kt + 1) * B]
                    wk = w_t[:, kt * F:(kt + 1) * F]
                    nc.tensor.matmul(
                        ps[:, 0:512], lhsT, wk[:, 0:512],
                        start=(kt == 0), stop=(kt == KT - 1), skip_group_check=True,
                    )
                    nc.tensor.matmul(
                        ps[:, 512:1024], lhsT, wk[:, 512:F],
                        start=(kt == 0), stop=(kt == KT - 1), skip_group_check=True,
                    )

                # evacuate PSUM + add bias, packing 4 channels per [B, 4F]
                # out tile -> 8 KB/partition output descriptors. Two 512-col
                # ops so the first can overlap the second half's last matmuls
                if c % 4 == 0:
                    o_t = opool.tile([B, 4 * F], FP16 if c < 8 else FP8)
                osc = OSCALE if c < 8 else OSCALE * 2.0
                goff = (c % 4) * F
                nc.vector.scalar_tensor_tensor(
                    o_t[:, goff:goff + 512], ps[:, 0:512], osc,
                    bb[:, c * F:c * F + 512],
                    op0=mybir.AluOpType.mult, op1=mybir.AluOpType.add,
                )
                nc.vector.scalar_tensor_tensor(
                    o_t[:, goff + 512:goff + F], ps[:, 512:1024], osc,
                    bb[:, c * F + 512:(c + 1) * F],
                    op0=mybir.AluOpType.mult, op1=mybir.AluOpType.add,
                )
                if c // 4 < 3:
                    if c % 4 == 3:
                        # scalar engine's HW-DGE queue: fast, and off the
                        # sync-engine queues that stream W
                        yd = yc16[c // 4] if c < 8 else yc8[c // 4 - 2]
                        nc.scalar.dma_start(yd, o_t[:])
                else:
                    # last group: progressively finer flushes so the final
                    # DMA is small and fires right after ch15's evacuation
                    if c == 13:
                        nc.scalar.dma_start(yc8[1][:, 0:2 * F], o_t[:, 0:2 * F])
                    elif c == 14:
                        nc.scalar.dma_start(yc8[1][:, 2 * F:3 * F], o_t[:, 2 * F:3 * F])
                    elif c == 15:
                        # final flush in 512-col pieces: the first fires
                        # while ps half 1 is still being evacuated, and the
                        # very last DMA is tiny so it retires fast
                        nc.scalar.dma_start(
                            yc8[1][:, 3 * F:3 * F + 512], o_t[:, 3 * F:3 * F + 512]
                        )
                        nc.scalar.dma_start(
                            yc8[1][:, 3 * F + 512:4 * F], o_t[:, 3 * F + 512:4 * F]
                        )

    nc.compile()
    _CACHE["nc"] = nc
    return nc


def shard_inputs(x, W, b):
    f8 = ml_dtypes.float8_e3m4
    in_maps = []
    for core in range(NCORES):
        cs, ce = core * CPC, (core + 1) * CPC
        # wh[c, p, kt*F + g] = 32 * W[c][g][kt*128 + p]
        wt = (W[cs:ce].astype(np.float32) * WSCALE).astype(f8).transpose(0, 2, 1)
        wh = np.ascontiguousarray(
            wt.reshape(CPC, KT, 128, F).transpose(0, 2, 1, 3)
        ).reshape(CPC, 128, KT * F)
        xt = (x[:, cs:ce, :].astype(np.float32) * XSCALE).astype(f8)
        xt = xt.transpose(1, 2, 0)                            # [CPC, f, b]
        xs = np.ascontiguousarray(
            xt.reshape(CPC, KT, 128, B).transpose(2, 0, 1, 3)
        ).reshape(128, CPC * KT * B)
        bsc = b[cs:ce].astype(np.float32).copy()
        bsc[CPC // 2:] *= 2.0
        bsh = np.ascontiguousarray(bsc.reshape(1, CPC * F).astype(np.float16))
        in_maps.append({"wh": wh, "xs": xs, "bs": bsh})
    return in_maps


def gather_output(results):
    y16 = np.stack([results[core]["yc16"] for core in range(NCORES)])
    y8 = np.stack([results[core]["yc8"] for core in range(NCORES)])
    yc = np.concatenate(
        [y16.astype(np.float32), y8.astype(np.float32) / 2.0], axis=1
    )                                                   # [core, q, b, 4F]
    # channel = core*CPC + q*4 + j, cols j*F+g
    y = yc.reshape(NCORES, CPC // 4, B, 4, F)           # [core, q, b, j, g]
    y = y.transpose(0, 1, 3, 2, 4).reshape(C, B, F)     # [c, b, g]
    return np.ascontiguousarray(y.transpose(1, 0, 2).astype(np.float32))


def kernel(x, W, b):
    x = np.asarray(x)
    W = np.asarray(W)
    b = np.asarray(b)
    nc = _build()
    in_maps = shard_inputs(x, W, b)
    res = bass_utils.run_bass_kernel_spmd(nc, in_maps, core_ids=list(range(NCORES)))
    return gather_output(res.results)


# revision 33
# speedup vs baseline: 1.0154x; 1.0154x over previous
"""Channel-wise Linear on 8 TRN2 NeuronCores.

y[b, c, :] = x[b, c, :] @ W[c].T + b[c]   (B=64, C=128, F=1024, fp32 ref)

Sharding: channels split across 8 cores (16 each, expert-style), no
cross-core communication.

Numerics: traffic is the whole story (W would be 32 MB/core in bf16), so
both matmul operands ship as float8 e3m4 (4 mantissa bits): W scaled by
32 and x by 2 so the randn-scaled values sit in the e3m4 normal range
(max 15.5, no clipping). PSUM accumulates in fp32; the 1/64 rescale
rides the scalar slot of the PSUM-evacuation op. Channels 0-7 of each
core leave as fp16, channels 8-15 as e3m4 at 2x scale (host halves
them back; their bias ships pre-doubled). Measured end-to-end rel err
1.863e-2 vs the 2e-2 gate - deterministic (fixed seed + deterministic
quantization; numpy simulation matches HW exactly every run). e4m3's
3-bit mantissa would fail at 2.3e-2.

Device, per channel: 8 K-tiles of xT.T @ WT accumulate into one 2-bank
PSUM tile (x stationary, W moving at 1 col/cycle; LDWEIGHTS of the next
k-tile overlaps the current matmul, verified on HW). Bias never touches
the PE: bias rows are partition-broadcast on gpsimd, and PSUM->SBUF
evacuation is a fused (psum*1/64 + bias) scalar_tensor_tensor on the
vector engine - two 512-col ops so the first overlaps the second half's
last matmuls - packing 4 channels side by side into a [64, 4F] fp16
tile (8 KB/partition descriptors). Outputs go out on the scalar
engine's HW-DGE queue, off the 16 sync-engine queues that stream W;
the last group flushes in progressively finer pieces so the final DMA
is small and fires immediately after channel 15's evacuation.

Per-core traffic: 16.8 MB W + 1.05 MB x + 1.6 MB y ~ 19.5 MB (~54 us at
the ~360 GB/s/core the 16 shared DMA engines sustain), PE: 16ch x 8kt
x 1024 cols = 131k cycles (~55 us at 2.4 GHz) - right at the ridge, so
exec is DMA-paced and run-to-run HBM contention shows as +-4 us.

Schedule notes (from perfetto/NTFF traces):
- HAM clock-gates the PE to 4/8 until ~4 us of SUSTAINED wide matmul
  activity; tiny seed matmuls don't ramp it, and a >~2.5 us idle drops
  it back. The warm-up chain of full-array matmuls is sized to end just
  as channel 0's first W chunk lands (~13 us), handing off seamlessly.
- DMA takes ~7 us after kernel start to begin flowing (engine-boot
  barriers), then ~5 us to ramp to full rate; channel 0's x chunk and a
  small leading W chunk are enqueued first so kt0 starts earliest.
- The last ~6 us is NEFF teardown (per-engine event-semaphore resets +
  two cross-core barriers) emitted by codegen - not addressable from
  kernel code; the Tile-context sem-clear chain is patched out below.
"""

import numpy as np
import ml_dtypes

import concourse.bass as bass
import concourse.bacc as bacc
import concourse.mybir as mybir
from concourse import tile
from concourse import bass_utils

B, C, F = 64, 128, 1024
NCORES = 8
CPC = C // NCORES          # channels per core
KT = F // 128              # contraction tiles per channel
F32 = mybir.dt.float32
BF16 = mybir.dt.bfloat16
FP16 = mybir.dt.float16
FP8 = mybir.dt.float8e3    # e3m4
WSCALE = 32.0
XSCALE = 2.0
OSCALE = 1.0 / (WSCALE * XSCALE)
WARMUP_MM = 15             # warm-up matmuls (~0.4-0.8 us each while ramping)

_CACHE = {}


def _patch_fast_teardown():
    """Skip the end-of-kernel semaphore-clear chain (~5-8 us of serial
    EVENT_SEMAPHORE resets + an extra all-engine barrier). The NEFF here is
    loaded fresh per execution, so post-run semaphore hygiene only adds tail
    latency. Allocator bookkeeping (freeing the sem ids) is preserved by
    calling the original helper with instruction emission suppressed."""
    from concourse import tile as _t

    if getattr(_t.TileContext._drain_and_barrier, "_fast_teardown", False):
        return

    def _drain_and_barrier(self, tick_clock, wait_clock):
        drain_inst = self.nc.sync.drain()
        wait_clock.add_sem_waits(
            drain_inst.ins, _t.ScopedClock({None: tick_clock.global_clock})
        )
        self.nc.all_engine_barrier()
        assert self.sems is not None
        popped = self.nc._tile_sem_poison_stack.pop()
        assert popped is self._sem_poison
        gp = self.nc.gpsimd
        orig = (gp.dma_reset, gp.sem_clear)
        try:
            gp.dma_reset = lambda *a, **k: None
            gp.sem_clear = lambda *a, **k: None
        except AttributeError:
            self.nc.clear_and_free_semaphores(list(self.sems.allocated().values()))
            self.nc.all_engine_barrier()
            return
        try:
            self.nc.clear_and_free_semaphores(list(self.sems.allocated().values()))
        finally:
            gp.dma_reset, gp.sem_clear = orig

    _drain_and_barrier._fast_teardown = True
    _t.TileContext._drain_and_barrier = _drain_and_barrier


def _build():
    if "nc" in _CACHE:
        return _CACHE["nc"]
    _patch_fast_teardown()
    nc = bacc.Bacc(
        "TRN2",
        target_bir_lowering=False,
        debug=False,
        enable_asserts=False,
        num_devices=NCORES,
    )
    wh = nc.dram_tensor("wh", [CPC, 128, KT * F], FP8, kind="ExternalInput").ap()
    xs = nc.dram_tensor("xs", [128, CPC * KT * B], FP8, kind="ExternalInput").ap()
    bs = nc.dram_tensor("bs", [1, CPC * F], FP16, kind="ExternalInput").ap()
    # channels 0-7 leave as fp16; channels 8-15 as fp8 e3m4 at 2x scale
    # (host halves them back) - rel err 1.86e-2 vs the 2e-2 gate, and 0.5 MB
    # less output traffic per core
    yc16 = nc.dram_tensor("yc16", [2, B, 4 * F], FP16, kind="ExternalOutput").ap()
    yc8 = nc.dram_tensor("yc8", [2, B, 4 * F], FP8, kind="ExternalOutput").ap()

    with tile.TileContext(nc) as tc:
        with (
            tc.tile_pool(name="w", bufs=12) as wpool,
            tc.tile_pool(name="x", bufs=1) as xpool,
            tc.tile_pool(name="bi", bufs=1) as bpool,
            tc.tile_pool(name="wa", bufs=1) as wupool,
            tc.tile_pool(name="o", bufs=3) as opool,
            tc.tile_pool(name="ps", bufs=4, space=bass.MemorySpace.PSUM) as pspool,
        ):
            # PE warm-up: full-array matmuls on junk data keep the PE busy
            # while channel 0's W streams in; HAM only ramps the PE clock to
            # 8/8 after ~6 us of SUSTAINED wide matmul activity (tiny seed
            # matmuls don't ramp it), so make the warm-up look like real work.
            wa = wupool.tile([128, 512], BF16)
            nc.gpsimd.memset(wa[:], 1.0)
            wu = pspool.tile([128, 512], F32, tag="ps")
            for _ in range(WARMUP_MM):
                nc.tensor.matmul(
                    wu[:], wa[:, 0:128], wa[:], start=True, stop=True,
                    skip_group_check=True,
                )

            b_sb = bpool.tile([1, CPC * F], FP16)
            bb = bpool.tile([B, CPC * F], FP16)

            # x prefetched one 4-channel group ahead of its W stream so W
            # never queues behind bulk x traffic on the HW-DGE queues
            x_all = xpool.tile([128, CPC * KT * B], FP8)
            xq = CPC * KT * B // 4

            o_t = None
            for c in range(CPC):
                w_t = wpool.tile([128, KT * F], FP8)
                half = KT * F // 2
                if c == 0:
                    # channel 0 owns the critical path: its own tiny x chunk
                    # and a small leading W chunk first so kt0 starts early
                    xc0 = KT * B
                    nc.sync.dma_start(x_all[:, 0:xc0], xs[:, 0:xc0])
                    qf = 2 * F
                    nc.sync.dma_start(w_t[:, 0:qf], wh[c][:, 0:qf])
                    nc.sync.dma_start(x_all[:, xc0:xq], xs[:, xc0:xq])
                    nc.sync.dma_start(w_t[:, qf:half], wh[c][:, qf:half])
                    nc.sync.dma_start(w_t[:, half:], wh[c][:, half:])
                    nc.sync.dma_start(b_sb[:], bs[:])
                elif c < 4 or c >= 12:
                    # fine-granularity W where the PE runs close behind the
                    # DMA: early channels (pipe still building its lead) and
                    # late channels (in DMA-paced phases the tail otherwise
                    # waits on the whole final 1 MB before any matmul)
                    nq = 2 if c < 14 else 4
                    st = KT * F // nq
                    for j in range(nq):
                        nc.sync.dma_start(
                            w_t[:, j * st:(j + 1) * st], wh[c][:, j * st:(j + 1) * st]
                        )
                else:
                    # steady state: one dma_start per channel (8 KB/partition
                    # descriptors), prefetch pool hides the coarser signal
                    nc.sync.dma_start(w_t[:], wh[c])
                # bias rows broadcast over the batch partitions on gpsimd
                nc.gpsimd.partition_broadcast(
                    bb[:, c * F:(c + 1) * F], b_sb[:, c * F:(c + 1) * F], channels=B
                )
                if c % 4 == 0 and c + 4 < CPC:
                    j = c // 4 + 1
                    nc.sync.dma_start(
                        x_all[:, j * xq:(j + 1) * xq], xs[:, j * xq:(j + 1) * xq]
                    )

                # one 2-bank PSUM tile per channel: a single 1024-col
                # evacuation op instead of two
                ps = pspool.tile([B, 1024], F32, tag="ps")
                for kt in range(KT):
                    lhsT = x_all[:, (c * KT + kt) * B:(c * KT + kt + 1) * B]
                    wk = w_t[:, kt * F:(kt + 1) * F]
                    nc.tensor.matmul(
                        ps[:, 0:512], lhsT, wk[:, 0:512],
                        start=(kt == 0), stop=(kt == KT - 1), skip_group_check=True,
                    )
                    nc.tensor.matmul(
                        ps[:, 512:1024], lhsT, wk[:, 512:F],
                        start=(kt == 0), stop=(kt == KT - 1), skip_group_check=True,
                    )

                # evacuate PSUM + add bias, packing 4 channels per [B, 4F]
                # out tile -> 8 KB/partition output descriptors. Two 512-col
                # ops so the first can overlap the second half's last matmuls
                if c % 4 == 0:
                    o_t = opool.tile([B, 4 * F], FP16 if c < 8 else FP8)
                osc = OSCALE if c < 8 else OSCALE * 2.0
                goff = (c % 4) * F
                nc.vector.scalar_tensor_tensor(
                    o_t[:, goff:goff + 512], ps[:, 0:512], osc,
                    bb[:, c * F:c * F + 512],
                    op0=mybir.AluOpType.mult, op1=mybir.AluOpType.add,
                )
                nc.vector.scalar_tensor_tensor(
                    o_t[:, goff + 512:goff + F], ps[:, 512:1024], osc,
                    bb[:, c * F + 512:(c + 1) * F],
                    op0=mybir.AluOpType.mult, op1=mybir.AluOpType.add,
                )
                if c // 4 < 3:
                    if c % 4 == 3:
                        # scalar engine's HW-DGE queue: fast, and off the
                        # sync-engine queues that stream W
                        yd = yc16[c // 4] if c < 8 else yc8[c // 4 - 2]
                        nc.scalar.dma_start(yd, o_t[:])
                else:
                    # last group: progressively finer flushes so the final
                    # DMA is small and fires right after ch15's evacuation
                    if c == 13:
                        nc.scalar.dma_start(yc8[1][:, 0:2 * F], o_t[:, 0:2 * F])
                    elif c == 14:
                        nc.scalar.dma_start(yc8[1][:, 2 * F:3 * F], o_t[:, 2 * F:3 * F])
                    elif c == 15:
                        # final flush in 512-col pieces: the first fires
                        # while ps half 1 is still being evacuated, and the
                        # very last DMA is tiny so it retires fast
                        nc.scalar.dma_start(
                            yc8[1][:, 3 * F:3 * F + 512], o_t[:, 3 * F:3 * F + 512]
                        )
                        nc.scalar.dma_start(
                            yc8[1][:, 3 * F + 512:4 * F], o_t[:, 3 * F + 512:4 * F]
                        )

    nc.compile()
    _CACHE["nc"] = nc
    return nc


def shard_inputs(x, W, b):
    f8 = ml_dtypes.float8_e3m4
    in_maps = []
    for core in range(NCORES):
        cs, ce = core * CPC, (core + 1) * CPC
        # wh[c, p, kt*F + g] = 32 * W[c][g][kt*128 + p]
        wt = (W[cs:ce].astype(np.float32) * WSCALE).astype(f8).transpose(0, 2, 1)
        wh = np.ascontiguousarray(
            wt.reshape(CPC, KT, 128, F).transpose(0, 2, 1, 3)
        ).reshape(CPC, 128, KT * F)
        xt = (x[:, cs:ce, :].astype(np.float32) * XSCALE).astype(f8)
        xt = xt.transpose(1, 2, 0)                            # [CPC, f, b]
        xs = np.ascontiguousarray(
            xt.reshape(CPC, KT, 128, B).transpose(2, 0, 1, 3)
        ).reshape(128, CPC * KT * B)
        bsc = b[cs:ce].astype(np.float32).copy()
        bsc[CPC // 2:] *= 2.0
        bsh = np.ascontiguousarray(bsc.reshape(1, CPC * F).astype(np.float16))
        in_maps.append({"wh": wh, "xs": xs, "bs": bsh})
    return in_maps


def gather_output(results):
    y16 = np.stack([results[core]["yc16"] for core in range(NCORES)])
    y8 = np.stack([results[core]["yc8"] for core in range(NCORES)])
    yc = np.concatenate(
        [y16.astype(np.float32), y8.astype(np.float32) / 2.0], axis=1
    )                                                   # [core, q, b, 4F]
    # channel = core*CPC + q*4 + j, cols j*F+g
    y = yc.reshape(NCORES, CPC // 4, B, 4, F)           # [core, q, b, j, g]
    y = y.transpose(0, 1, 3, 2, 4).reshape(C, B, F)     # [c, b, g]
    return np.ascontiguousarray(y.transpose(1, 0, 2).astype(np.float32))


def kernel(x, W, b):
    x = np.asarray(x)
    W = np.asarray(W)
    b = np.asarray(b)
    nc = _build()
    in_maps = shard_inputs(x, W, b)
    res = bass_utils.run_bass_kernel_spmd(nc, in_maps, core_ids=list(range(NCORES)))
    return gather_output(res.results)


# revision 34
# speedup vs baseline: 1.0223x; 1.0068x over previous
"""Channel-wise Linear on 8 TRN2 NeuronCores.

y[b, c, :] = x[b, c, :] @ W[c].T + b[c]   (B=64, C=128, F=1024, fp32 ref)

Sharding: channels split across 8 cores (16 each, expert-style), no
cross-core communication.

Numerics: traffic is the whole story (W would be 32 MB/core in bf16), so
both matmul operands ship as float8 e3m4 (4 mantissa bits): W scaled by
32 and x by 2 so the randn-scaled values sit in the e3m4 normal range
(max 15.5, no clipping). PSUM accumulates in fp32; the 1/64 rescale
rides the scalar slot of the PSUM-evacuation op. Channels 0-7 of each
core leave as fp16, channels 8-15 as e3m4 at 2x scale (host halves
them back; their bias ships pre-doubled). Measured end-to-end rel err
1.863e-2 vs the 2e-2 gate - deterministic (fixed seed + deterministic
quantization; numpy simulation matches HW exactly every run). e4m3's
3-bit mantissa would fail at 2.3e-2.

Device, per channel: 8 K-tiles of xT.T @ WT accumulate into one 2-bank
PSUM tile (x stationary, W moving at 1 col/cycle; LDWEIGHTS of the next
k-tile overlaps the current matmul, verified on HW). Bias never touches
the PE: bias rows are partition-broadcast on gpsimd, and PSUM->SBUF
evacuation is a fused (psum*1/64 + bias) scalar_tensor_tensor on the
vector engine - two 512-col ops so the first overlaps the second half's
last matmuls - packing 4 channels side by side into a [64, 4F] fp16
tile (8 KB/partition descriptors). Outputs go out on the scalar
engine's HW-DGE queue, off the 16 sync-engine queues that stream W;
the last group flushes in progressively finer pieces so the final DMA
is small and fires immediately after channel 15's evacuation.

Per-core traffic: 16.8 MB W + 1.05 MB x + 1.6 MB y ~ 19.5 MB (~54 us at
the ~360 GB/s/core the 16 shared DMA engines sustain), PE: 16ch x 8kt
x 1024 cols = 131k cycles (~55 us at 2.4 GHz) - right at the ridge, so
exec is DMA-paced and run-to-run HBM contention shows as +-4 us.

Schedule notes (from perfetto/NTFF traces):
- HAM clock-gates the PE to 4/8 until ~4 us of SUSTAINED wide matmul
  activity; tiny seed matmuls don't ramp it, and a >~2.5 us idle drops
  it back. The warm-up chain of full-array matmuls is sized to end just
  as channel 0's first W chunk lands (~13 us), handing off seamlessly.
- DMA takes ~7 us after kernel start to begin flowing (engine-boot
  barriers), then ~5 us to ramp to full rate; channel 0's x chunk and a
  small leading W chunk are enqueued first so kt0 starts earliest.
- The last ~6 us is NEFF teardown (per-engine event-semaphore resets +
  two cross-core barriers) emitted by codegen - not addressable from
  kernel code; the Tile-context sem-clear chain is patched out below.
"""

import numpy as np
import ml_dtypes

import concourse.bass as bass
import concourse.bacc as bacc
import concourse.mybir as mybir
from concourse import tile
from concourse import bass_utils

B, C, F = 64, 128, 1024
NCORES = 8
CPC = C // NCORES          # channels per core
KT = F // 128              # contraction tiles per channel
F32 = mybir.dt.float32
BF16 = mybir.dt.bfloat16
FP16 = mybir.dt.float16
FP8 = mybir.dt.float8e3    # e3m4
WSCALE = 32.0
XSCALE = 2.0
OSCALE = 1.0 / (WSCALE * XSCALE)
WARMUP_MM = 13             # warm-up matmuls (~0.4-0.8 us each while ramping)

_CACHE = {}


def _patch_fast_teardown():
    """Skip the end-of-kernel semaphore-clear chain (~5-8 us of serial
    EVENT_SEMAPHORE resets + an extra all-engine barrier). The NEFF here is
    loaded fresh per execution, so post-run semaphore hygiene only adds tail
    latency. Allocator bookkeeping (freeing the sem ids) is preserved by
    calling the original helper with instruction emission suppressed."""
    from concourse import tile as _t

    if getattr(_t.TileContext._drain_and_barrier, "_fast_teardown", False):
        return

    def _drain_and_barrier(self, tick_clock, wait_clock):
        drain_inst = self.nc.sync.drain()
        wait_clock.add_sem_waits(
            drain_inst.ins, _t.ScopedClock({None: tick_clock.global_clock})
        )
        self.nc.all_engine_barrier()
        assert self.sems is not None
        popped = self.nc._tile_sem_poison_stack.pop()
        assert popped is self._sem_poison
        gp = self.nc.gpsimd
        orig = (gp.dma_reset, gp.sem_clear)
        try:
            gp.dma_reset = lambda *a, **k: None
            gp.sem_clear = lambda *a, **k: None
        except AttributeError:
            self.nc.clear_and_free_semaphores(list(self.sems.allocated().values()))
            self.nc.all_engine_barrier()
            return
        try:
            self.nc.clear_and_free_semaphores(list(self.sems.allocated().values()))
        finally:
            gp.dma_reset, gp.sem_clear = orig

    _drain_and_barrier._fast_teardown = True
    _t.TileContext._drain_and_barrier = _drain_and_barrier


def _build():
    if "nc" in _CACHE:
        return _CACHE["nc"]
    _patch_fast_teardown()
    nc = bacc.Bacc(
        "TRN2",
        target_bir_lowering=False,
        debug=False,
        enable_asserts=False,
        num_devices=NCORES,
    )
    wh = nc.dram_tensor("wh", [CPC, 128, KT * F], FP8, kind="ExternalInput").ap()
    xs = nc.dram_tensor("xs", [128, CPC * KT * B], FP8, kind="ExternalInput").ap()
    bs = nc.dram_tensor("bs", [1, CPC * F], FP16, kind="ExternalInput").ap()
    # channels 0-7 leave as fp16; channels 8-15 as fp8 e3m4 at 2x scale
    # (host halves them back) - rel err 1.86e-2 vs the 2e-2 gate, and 0.5 MB
    # less output traffic per core
    yc16 = nc.dram_tensor("yc16", [2, B, 4 * F], FP16, kind="ExternalOutput").ap()
    yc8 = nc.dram_tensor("yc8", [2, B, 4 * F], FP8, kind="ExternalOutput").ap()

    with tile.TileContext(nc) as tc:
        with (
            tc.tile_pool(name="w", bufs=12) as wpool,
            tc.tile_pool(name="x", bufs=1) as xpool,
            tc.tile_pool(name="bi", bufs=1) as bpool,
            tc.tile_pool(name="wa", bufs=1) as wupool,
            tc.tile_pool(name="o", bufs=3) as opool,
            tc.tile_pool(name="ps", bufs=4, space=bass.MemorySpace.PSUM) as pspool,
        ):
            # PE warm-up: full-array matmuls on junk data keep the PE busy
            # while channel 0's W streams in; HAM only ramps the PE clock to
            # 8/8 after ~6 us of SUSTAINED wide matmul activity (tiny seed
            # matmuls don't ramp it), so make the warm-up look like real work.
            wa = wupool.tile([128, 512], BF16)
            nc.gpsimd.memset(wa[:], 1.0)
            wu = pspool.tile([128, 512], F32, tag="ps")
            for _ in range(WARMUP_MM):
                nc.tensor.matmul(
                    wu[:], wa[:, 0:128], wa[:], start=True, stop=True,
                    skip_group_check=True,
                )

            b_sb = bpool.tile([1, CPC * F], FP16)
            bb = bpool.tile([B, CPC * F], FP16)

            # x prefetched one 4-channel group ahead of its W stream so W
            # never queues behind bulk x traffic on the HW-DGE queues
            x_all = xpool.tile([128, CPC * KT * B], FP8)
            xq = CPC * KT * B // 4

            o_t = None
            for c in range(CPC):
                w_t = wpool.tile([128, KT * F], FP8)
                half = KT * F // 2
                if c == 0:
                    # channel 0 owns the critical path: its own tiny x chunk
                    # and a small leading W chunk first so kt0 starts early
                    xc0 = KT * B
                    nc.sync.dma_start(x_all[:, 0:xc0], xs[:, 0:xc0])
                    qf = F
                    nc.sync.dma_start(w_t[:, 0:qf], wh[c][:, 0:qf])
                    nc.sync.dma_start(x_all[:, xc0:xq], xs[:, xc0:xq])
                    nc.sync.dma_start(w_t[:, qf:half], wh[c][:, qf:half])
                    nc.sync.dma_start(w_t[:, half:], wh[c][:, half:])
                    nc.sync.dma_start(b_sb[:], bs[:])
                elif c < 6 or c >= 12:
                    # fine-granularity W where the PE runs close behind the
                    # DMA: early channels (pipe still building its lead) and
                    # late channels (in DMA-paced phases the tail otherwise
                    # waits on the whole final 1 MB before any matmul)
                    nq = 2 if c < 14 else 4
                    st = KT * F // nq
                    for j in range(nq):
                        nc.sync.dma_start(
                            w_t[:, j * st:(j + 1) * st], wh[c][:, j * st:(j + 1) * st]
                        )
                else:
                    # steady state: one dma_start per channel (8 KB/partition
                    # descriptors), prefetch pool hides the coarser signal
                    nc.sync.dma_start(w_t[:], wh[c])
                # bias rows broadcast over the batch partitions on gpsimd
                nc.gpsimd.partition_broadcast(
                    bb[:, c * F:(c + 1) * F], b_sb[:, c * F:(c + 1) * F], channels=B
                )
                if c % 4 == 0 and c + 4 < CPC:
                    j = c // 4 + 1
                    nc.sync.dma_start(
                        x_all[:, j * xq:(j + 1) * xq], xs[:, j * xq:(j + 1) * xq]
                    )

                # one 2-bank PSUM tile per channel: a single 1024-col
                # evacuation op instead of two
                ps = pspool.tile([B, 1024], F32, tag="ps")
                for kt in range(KT):
                    lhsT = x_all[:, (c * KT + kt) * B:(c * KT + kt + 1) * B]
                    wk = w_t[:, kt * F:(kt + 1) * F]
                    nc.tensor.matmul(
                        ps[:, 0:512], lhsT, wk[:, 0:512],
                        start=(kt == 0), stop=(kt == KT - 1), skip_group_check=True,
                    )
                    nc.tensor.matmul(
                        ps[:, 512:1024], lhsT, wk[:, 512:F],
                        start=(kt == 0), stop=(kt == KT - 1), skip_group_check=True,
                    )

                # evacuate PSUM + add bias, packing 4 channels per [B, 4F]
                # out tile -> 8 KB/partition output descriptors. Two 512-col
                # ops so the first can overlap the second half's last matmuls
                if c % 4 == 0:
                    o_t = opool.tile([B, 4 * F], FP16 if c < 8 else FP8)
                osc = OSCALE if c < 8 else OSCALE * 2.0
                goff = (c % 4) * F
                nc.vector.scalar_tensor_tensor(
                    o_t[:, goff:goff + 512], ps[:, 0:512], osc,
                    bb[:, c * F:c * F + 512],
                    op0=mybir.AluOpType.mult, op1=mybir.AluOpType.add,
                )
                nc.vector.scalar_tensor_tensor(
                    o_t[:, goff + 512:goff + F], ps[:, 512:1024], osc,
                    bb[:, c * F + 512:(c + 1) * F],
                    op0=mybir.AluOpType.mult, op1=mybir.AluOpType.add,
                )
                if c // 4 < 3:
                    if c % 4 == 3:
                        # scalar engine's HW-DGE queue: fast, and off the
                        # sync-engine queues that stream W
                        yd = yc16[c // 4] if c < 8 else yc8[c // 4 - 2]
                        nc.scalar.dma_start(yd, o_t[:])
                else:
                    # last group: progressively finer flushes so the final
                    # DMA is small and fires right after ch15's evacuation
                    if c == 13:
                        nc.scalar.dma_start(yc8[1][:, 0:2 * F], o_t[:, 0:2 * F])
                    elif c == 14:
                        nc.scalar.dma_start(yc8[1][:, 2 * F:3 * F], o_t[:, 2 * F:3 * F])
                    elif c == 15:
                        # final flush in 512-col pieces: the first fires
                        # while ps half 1 is still being evacuated, and the
                        # very last DMA is tiny so it retires fast
                        nc.scalar.dma_start(
                            yc8[1][:, 3 * F:3 * F + 512], o_t[:, 3 * F:3 * F + 512]
                        )
                        nc.scalar.dma_start(
                            yc8[1][:, 3 * F + 512:4 * F], o_t[:, 3 * F + 512:4 * F]
                        )

    nc.compile()
    _CACHE["nc"] = nc
    return nc


def shard_inputs(x, W, b):
    f8 = ml_dtypes.float8_e3m4
    in_maps = []
    for core in range(NCORES):
        cs, ce = core * CPC, (core + 1) * CPC
        # wh[c, p, kt*F + g] = 32 * W[c][g][kt*128 + p]
        wt = (W[cs:ce].astype(np.float32) * WSCALE).astype(f8).transpose(0, 2, 1)
        wh = np.ascontiguousarray(
            wt.reshape(CPC, KT, 128, F).transpose(0, 2, 1, 3)
        ).reshape(CPC, 128, KT * F)
        xt = (x[:, cs:ce, :].astype(np.float32) * XSCALE).astype(f8)
        xt = xt.transpose(1, 2, 0)                            # [CPC, f, b]
        xs = np.ascontiguousarray(
            xt.reshape(CPC, KT, 128, B).transpose(2, 0, 1, 3)
        ).reshape(128, CPC * KT * B)
        bsc = b[cs:ce].astype(np.float32).copy()
        bsc[CPC // 2:] *= 2.0
        bsh = np.ascontiguousarray(bsc.reshape(1, CPC * F).astype(np.float16))
        in_maps.append({"wh": wh, "xs": xs, "bs": bsh})
    return in_maps


def gather_output(results):
    y16 = np.stack([results[core]["yc16"] for core in range(NCORES)])
    y8 = np.stack([results[core]["yc8"] for core in range(NCORES)])
    yc = np.concatenate(
        [y16.astype(np.float32), y8.astype(np.float32) / 2.0], axis=1
    )                                                   # [core, q, b, 4F]
    # channel = core*CPC + q*4 + j, cols j*F+g
    y = yc.reshape(NCORES, CPC // 4, B, 4, F)           # [core, q, b, j, g]
    y = y.transpose(0, 1, 3, 2, 4).reshape(C, B, F)     # [c, b, g]
    return np.ascontiguousarray(y.transpose(1, 0, 2).astype(np.float32))


def kernel(x, W, b):
    x = np.asarray(x)
    W = np.asarray(W)
    b = np.asarray(b)
    nc = _build()
    in_maps = shard_inputs(x, W, b)
    res = bass_utils.run_bass_kernel_spmd(nc, in_maps, core_ids=list(range(NCORES)))
    return gather_output(res.results)
